# revision 6
# baseline (speedup 1.0000x reference)
"""CrossAttention kernel for 8 trn2 NeuronCores (v4).

Reference:
  q = x @ Wq          [n, vq, h]
  k = y @ Wk          [n, vk, h]
  v = y @ Wv          [n, vk, c]
  out = softmax(q k^T / sqrt(h)) @ v        [n, vq, c]
with N=4, VQ=VK=4096, C=128, H=64, fp32.

Sharding: 8 cores = 4 batches x 2 query halves.

v4 (v1 154us, v2 135us, v3 103us):
  - all streamed inputs bf16 (xT, yT, pre-tiled y, weights): halves staging
    DMA; y arrives partition-major (one 8KB descriptor per partition).
  - projections in bf16 (K=c), scores matmuls f32r on the PSUM-cast q/k.
    No row duplication / tile_position: both chunk matmuls use rows 0:63.
  - per chunk (2 vk tiles): PE = 2 score streams + 2 z streams; ScalarE =
    one exp [128,1024] -> bf16 attn; VectorE = partial-sum accumulate.
    Softmax denominators: one ones-matmul per vq tile; host normalizes.
"""

import sys

sys.path.insert(0, "/opt/trn_rl_repo")

from contextlib import ExitStack

import ml_dtypes
import numpy as np

import concourse.bass as bass
import concourse.tile as tile
from concourse import mybir
from concourse.bass_utils import run_bass_kernel_spmd

F32 = mybir.dt.float32
F32R = mybir.dt.float32r
BF16 = mybir.dt.bfloat16
P = 128

N, VQ, VK, C, H = 4, 4096, 4096, 128, 64
VQ_PER = VQ // 2          # 2048 queries per core
SCALE = float(H) ** -0.5

VQ_T = 512                # vq tile (psum free dim)
N_VQ_T = VQ_PER // VQ_T   # 4
N_VK_T = VK // P          # 32 vk tiles of 128
CHUNK = 2                 # vk tiles per exp chunk
N_CH = N_VK_T // CHUNK    # 16 chunks per vq tile


def _split_multi_waits(nc):
    """walrus in this env supports one sync-wait per instruction; hoist
    extras onto same-engine NoOps inserted just before."""
    for fn in nc.m.functions:
        for bb in fn.blocks:
            out = []
            for inst in bb.instructions:
                si = inst.sync_info
                waits = list(si.on_wait) if si and si.on_wait else []
                if len(waits) > 1:
                    for w in waits[:-1]:
                        out.append(mybir.InstNoOp(
                            name=nc.get_next_instruction_name(),
                            engine=inst.engine,
                            ins=[], outs=[],
                            sync_info=mybir.SyncInfo(on_wait=[w], on_update=[]),
                        ))
                    inst.sync_info = mybir.SyncInfo(
                        on_wait=[waits[-1]],
                        on_update=list(si.on_update) if si.on_update else [],
                    )
                out.append(inst)
            bb.instructions = out


def _build():
    nc = bass.Bass()
    xT_d = nc.declare_dram_parameter("xT", [C, VQ_PER], BF16, isOutput=False)
    yT_d = nc.declare_dram_parameter("yT", [C, VK], BF16, isOutput=False)
    y_d = nc.declare_dram_parameter("y", [P, N_VK_T * P], BF16, isOutput=False)
    wq_d = nc.declare_dram_parameter("wq", [C, H], BF16, isOutput=False)
    wk_d = nc.declare_dram_parameter("wk", [C, H], BF16, isOutput=False)
    wv_d = nc.declare_dram_parameter("wv", [C, C], F32R, isOutput=False)
    oT_d = nc.declare_dram_parameter("oT", [C, VQ_PER], F32, isOutput=True)
    sums_d = nc.declare_dram_parameter("sums", [1, VQ_PER], F32, isOutput=True)

    with tile.TileContext(nc) as tc, ExitStack() as ctx:
        const = ctx.enter_context(tc.tile_pool(name="const", bufs=1))
        persist = ctx.enter_context(tc.tile_pool(name="persist", bufs=1))

        # ---- constants ----
        wqk_sb = const.tile([P, 2 * H], BF16)
        nc.sync.dma_start(wqk_sb[:, 0:H], wq_d[:])
        nc.sync.dma_start(wqk_sb[:, H:], wk_d[:])
        wv_sb = const.tile([P, C], F32R)
        nc.sync.dma_start(wv_sb[:], wv_d[:])
        ones_f = const.tile([P, 1], F32)
        nc.vector.memset(ones_f[:], 1.0)
        ones_b = const.tile([P, 1], BF16)
        nc.vector.tensor_copy(ones_b[:], ones_f[:])

        # ---- persistent tensors ----
        qT = persist.tile([64, VQ_PER], F32R)
        kT = persist.tile([64, VK], F32R)
        y_sb = persist.tile([P, N_VK_T, P], BF16)     # y tiles [vk, c]
        attn = persist.tile([P, N_VK_T * VQ_T], BF16)  # per vq tile, rotating
        acc = persist.tile([P, CHUNK * VQ_T], BF16)   # DVE partial sums
        acc2 = persist.tile([P, VQ_T], BF16)          # folded partial sums
        z_sb = persist.tile([P, VQ_PER], F32R)        # z = y^T attn  [c, vq]
        oT_sb = persist.tile([P, VQ_PER], F32)        # Wv^T z        [c, vq]
        srow = persist.tile([1, VQ_PER], F32)         # softmax sums

        with ExitStack() as mctx:
            # sc pool is also used (same tag) for projection bounces
            sc_ps = mctx.enter_context(
                tc.tile_pool(name="sc_ps", bufs=3, space="PSUM"))
            z_ps = mctx.enter_context(
                tc.tile_pool(name="z_ps", bufs=1, space="PSUM"))
            sm_ps = mctx.enter_context(
                tc.tile_pool(name="sm_ps", bufs=1, space="PSUM"))

            # staging SBUF for xT/yT chunk pairs (bf16 straight from DMA)
            stage = mctx.enter_context(tc.tile_pool(name="stage", bufs=1))
            qx_stage = [stage.tile([P, 1024], BF16, name=f"qx{i}")
                        for i in range(2)]
            ky_stage = [stage.tile([P, 1024], BF16, name=f"ky{i}")
                        for i in range(4)]

            # ---- staging: DMA + projections (emitted interleaved below) --
            def dma_x(chp):  # chp = chunk pair index (0..1), 1024 cols
                sl = slice(chp * 1024, (chp + 1) * 1024)
                nc.sync.dma_start(qx_stage[chp][:], xT_d[:, sl])

            def dma_yT(chp):  # chp 0..3, 1024 vk cols of yT
                sl = slice(chp * 1024, (chp + 1) * 1024)
                nc.sync.dma_start(ky_stage[chp][:], yT_d[:, sl])

            def dma_y_raw():  # pre-tiled [128, 32*128] bf16, partition-major
                nc.scalar.dma_start(
                    y_sb[:].rearrange("p t c -> p (t c)"), y_d[:])

            def proj_x(chp):
                ps = sc_ps.tile([64, 1024], F32, tag="sc", name=f"pjx{chp}")
                for b in range(2):
                    nc.tensor.matmul(
                        ps[:, b * 512:(b + 1) * 512], wqk_sb[:, 0:H],
                        qx_stage[chp][:, b * 512:(b + 1) * 512],
                        start=True, stop=True)
                sl = slice(chp * 1024, (chp + 1) * 1024)
                nc.vector.tensor_copy(qT[:, sl], ps[:])

            def proj_y(chp):
                ps = sc_ps.tile([64, 1024], F32, tag="sc", name=f"pjy{chp}")
                for b in range(2):
                    nc.tensor.matmul(
                        ps[:, b * 512:(b + 1) * 512], wqk_sb[:, H:],
                        ky_stage[chp][:, b * 512:(b + 1) * 512],
                        start=True, stop=True)
                sl = slice(chp * 1024, (chp + 1) * 1024)
                nc.vector.tensor_copy(kT[:, sl], ps[:])

            # ---- flash loop ----
            z_tiles = [None] * N_VQ_T

            def emit_scores_exp(j, c):
                sc = sc_ps.tile([P, CHUNK * VQ_T], F32, tag="sc")
                s = CHUNK * c
                for ii in range(CHUNK):
                    nc.tensor.matmul(
                        sc[:, ii * VQ_T:(ii + 1) * VQ_T],
                        kT[:, (s + ii) * P:(s + ii + 1) * P],
                        qT[:, j * VQ_T:(j + 1) * VQ_T],
                        start=True, stop=True)
                nc.scalar.activation(
                    attn[:, s * VQ_T:(s + 2) * VQ_T],
                    sc[:],
                    mybir.ActivationFunctionType.Exp, scale=SCALE)

            def emit_consume(j, c):
                if c == 0:
                    zp = z_ps.tile([P, VQ_T], F32, tag="z", name=f"z{j}")
                    z_tiles[j] = zp
                zp = z_tiles[j]
                for ii in range(CHUNK):
                    i = CHUNK * c + ii
                    a_sl = attn[:, i * VQ_T:(i + 1) * VQ_T]
                    nc.tensor.matmul(
                        zp[:], y_sb[:, i, :], a_sl,
                        start=(i == 0), stop=(i == N_VK_T - 1))
                # VectorE partial-sum accumulation (both tiles in one op)
                ch_sl = attn[:, CHUNK * c * VQ_T:CHUNK * (c + 1) * VQ_T]
                if c == 0:
                    nc.vector.tensor_copy(acc[:], ch_sl)
                else:
                    nc.vector.tensor_tensor(
                        out=acc[:], in0=acc[:], in1=ch_sl,
                        op=mybir.AluOpType.add)
                if c == N_CH - 1:
                    # fold halves, partition-reduce on PE, evacuate
                    nc.vector.tensor_tensor(
                        out=acc2[:], in0=acc[:, 0:VQ_T], in1=acc[:, VQ_T:],
                        op=mybir.AluOpType.add)
                    sm = sm_ps.tile([1, VQ_T], F32, tag="sm", name=f"sm{j}")
                    nc.tensor.matmul(sm[:], ones_b[:], acc2[:],
                                     start=True, stop=True)
                    nc.vector.tensor_copy(
                        z_sb[:, j * VQ_T:(j + 1) * VQ_T], zp[:])
                    nc.vector.tensor_copy(
                        srow[:, j * VQ_T:(j + 1) * VQ_T], sm[:])

            # background staging tasks spread over early flash chunks.
            pre = [lambda: dma_x(0), lambda: dma_yT(0), dma_y_raw,
                   lambda: proj_x(0), lambda: proj_y(0)]
            bg = [
                lambda: dma_yT(1),
                lambda: proj_y(1),          # kT for flash chunks 4..7
                lambda: dma_yT(2),
                lambda: dma_x(1),
                lambda: proj_y(2),          # flash 8..11
                lambda: dma_yT(3),
                lambda: proj_x(1),
                lambda: proj_y(3),          # flash 12..15
            ]
            for t in pre:
                t()

            work = [(j, c) for j in range(N_VQ_T) for c in range(N_CH)]
            for n, (j, c) in enumerate(work):
                emit_scores_exp(j, c)
                if n < len(bg):
                    bg[n]()
                if n > 0:
                    emit_consume(*work[n - 1])
            emit_consume(*work[-1])

        # ---- tail: oT = Wv^T z, store ----
        with ExitStack() as fctx:
            f_ps = fctx.enter_context(
                tc.tile_pool(name="f_ps", bufs=2, space="PSUM"))
            for j in range(N_VQ_T):
                sl = slice(j * VQ_T, (j + 1) * VQ_T)
                o2 = f_ps.tile([P, VQ_T], F32, tag="o2")
                nc.tensor.matmul(o2[:], wv_sb[:], z_sb[:, sl],
                                 start=True, stop=True)
                if j % 2:
                    nc.scalar.copy(oT_sb[:, sl], o2[:])
                else:
                    nc.vector.tensor_copy(oT_sb[:, sl], o2[:])
                nc.sync.dma_start(oT_d[:, sl], oT_sb[:, sl])
            nc.sync.dma_start(sums_d[:], srow[:])

    _split_multi_waits(nc)
    return nc


_NC = None


def _get_nc():
    global _NC
    if _NC is None:
        _NC = _build()
    return _NC


def make_in_maps(x, y, Wq, Wk, Wv):
    bf = ml_dtypes.bfloat16
    x = np.ascontiguousarray(x, dtype=np.float32)
    y = np.ascontiguousarray(y, dtype=np.float32)
    wq = np.ascontiguousarray(Wq, dtype=np.float32).astype(bf)
    wk = np.ascontiguousarray(Wk, dtype=np.float32).astype(bf)
    wv = np.ascontiguousarray(Wv, dtype=np.float32)
    in_maps = []
    for core in range(8):
        n, half = core // 2, core % 2
        yb = y[n].astype(bf)
        # pre-tiled, partition-major: [p, t, c] for vk row = 128*t + p
        y_tiled = np.ascontiguousarray(
            yb.reshape(N_VK_T, P, C).transpose(1, 0, 2).reshape(P, -1))
        in_maps.append({
            "xT": np.ascontiguousarray(
                x[n, half * VQ_PER:(half + 1) * VQ_PER, :].T).astype(bf),
            "yT": np.ascontiguousarray(y[n].T).astype(bf),
            "y": y_tiled,
            "wq": wq, "wk": wk, "wv": wv,
        })
    return in_maps


def finish(results):
    """Host-side epilogue: normalize + transpose per core shard."""
    out = np.empty((N, VQ, C), dtype=np.float32)
    for core in range(8):
        n, half = core // 2, core % 2
        r = results[core]
        out[n, half * VQ_PER:(half + 1) * VQ_PER, :] = (
            r["oT"] / r["sums"]).T
    return out


def kernel(x, y, Wq, Wk, Wv):
    nc = _get_nc()
    in_maps = make_in_maps(x, y, Wq, Wk, Wv)
    res = run_bass_kernel_spmd(nc, in_maps, list(range(8)))
    return finish(res.results)
